# revision 20
# baseline (speedup 1.0000x reference)
"""Trainium2 Bass kernel for the byte-LSTM autoregressive model.

Problem: B=64, T=8192, D=32 (LSTM state), C=256 output categories.
  xf = x/255 - 0.5
  h_in = shift_right(xf[:,:,None]*W_in[0] + b_in, bos)
  gx = h_in @ Wi + b_lstm                    (gates i,f,g,o)
  (c,h) LSTM scan over T steps with Wh
  out = hs @ W_out + b_out                   [B,T,C]

Sharding: data-parallel over batch, 8 sequences per core, 8 cores.

Exact algebraic folds (validated against the reference to ~3e-7 rel):
  * h_in is rank-1 in the scalar input, so gx[t] = v*xf[t-1] + w with
    v = Wi^T W_in[0], w = Wi^T b_in + b_lstm (g0 = Wi^T bos + b_lstm at t=0).
    These ride as 3 extra contraction rows of the recurrent matmul:
    W35 = [Wh_eff; v; w; g0] (K=35), per-step rhs = [H_{t-1}; xf; 1; t==0].
  * sigma(x) = 0.5*tanh(x/2)+0.5 -> ONE tanh activation covers all 4 gates
    (g-gate rows pre-doubled in the weights to undo the 0.5 scale).
  * Scaled state H = 2h, C = 2c. Per step (fp32):
      G   = W35^T [H_{t-1}; r3_t]         (2 matmuls, M=64 -> one PSUM tile)
      T   = tanh(0.5*G)                   (ACT, one [64,16] op)
      u   = (T_i + 1) * T_g               (DVE scalar_tensor_tensor)
      a   = (T_f + 1) * C_{t-1}           (DVE STT)
      C_t = 0.5*a + u                     (DVE STT)
      m   = tanh(0.5*C_t)                 (ACT)
      H_t = (T_o + 1) * m                 (DVE STT -> ring buffer)
    with Wh_eff = 0.5*Wh (g-cols x2), Wout_eff = 0.5*W_out.
    Gate layout / base partitions are arranged so every 2-input DVE op's
    SBUF inputs share a base partition (walrus NCC_IBIR297): the two
    matmuls write [i;f] and [g;o] into adjacent PSUM column blocks, so
    T[0:32,0:8]=t_i, T[32:64,0:8]=t_o... see build_bass.
  * H history lives in a double ring buffer (ring0 at partitions 0..34,
    ring1 at 64..98 — K=35 operands must sit at base partition 0 or 64).
    Each For_i iteration: 512 steps on ring0, 512 on ring1; column j holds
    [H_{t-1}; r3_t], i.e. the rhs of step t's matmul. The ring doubles as
    the (transposed) lhsT of the output projection, which is interleaved
    (one [128 rows, 256] block per 16 steps, bias pre-accumulated via a
    K=1 ones x b_out matmul).

The scan is a serial dependency chain (~6 engine hops per step); the
For_i wrapper (8 iterations) exists to reset semaphores at the back-edge
(a fully unrolled 8192-step scan overflows the 15-bit semaphore range:
4 DVE increments/step x 8192 = 32768) and to keep compile time sane.
"""

import os
import numpy as np

import concourse.bass as bass
import concourse.bacc as bacc
import concourse.mybir as mybir
import concourse.tile as tile
from concourse.bass import ds
from concourse.bass_utils import run_bass_kernel_spmd

F32 = mybir.dt.float32
AX = mybir.ActivationFunctionType
OP = mybir.AluOpType
ET = mybir.EngineType

B, T, D, C = 64, 8192, 32, 256
NCORES = 8
BS = B // NCORES          # batch per core = 8

# dev override for quick bring-up tests (number of time steps)
T_RUN = int(os.environ.get("KERNEL_T_OVERRIDE", T))
HALF = min(512, T_RUN // 2)        # steps per ring half
assert HALF % 16 == 0
assert T_RUN % (2 * HALF) == 0
ITERS = T_RUN // (2 * HALF)
# bench mode: constant DMA offsets (data repeats, timing identical) and an
# arbitrary iteration count so exec time rises above RPC noise
BENCH_ITERS = int(os.environ.get("KERNEL_BENCH_ITERS", "0"))
if BENCH_ITERS:
    ITERS = BENCH_ITERS
RCOLS = (HALF + 1) * BS            # ring element-columns (+1 tail col)
NBLK = HALF // 16                  # outproj blocks per half


def build_bass() -> bass.Bass:
    nc = bacc.Bacc("TRN2", target_bir_lowering=False, debug=False,
                   num_devices=NCORES)

    r3x = nc.dram_tensor("r3x", [3, T_RUN * BS], F32, kind="ExternalInput")
    w35 = nc.dram_tensor("w35", [35, 128], F32, kind="ExternalInput")
    wout = nc.dram_tensor("wout", [D, C], F32, kind="ExternalInput")
    bout = nc.dram_tensor("bout", [1, C], F32, kind="ExternalInput")
    ones = nc.dram_tensor("ones", [1, 128], F32, kind="ExternalInput")
    out = nc.dram_tensor("out", [BS, T_RUN, C], F32, kind="ExternalOutput")

    with tile.TileContext(nc) as tc:
        with (
            tc.tile_pool(name="const", bufs=1) as cpool,
            tc.tile_pool(name="gpsum", bufs=4, space="PSUM") as gpool,
            tc.tile_pool(name="opsum", bufs=2, space="PSUM") as opool,
            tc.tile_pool(name="tpool", bufs=4) as tpool,
            tc.tile_pool(name="spool", bufs=3) as spool,
            tc.tile_pool(name="mpool", bufs=3) as mpool,
            tc.tile_pool(name="obuf", bufs=2) as obpool,
        ):
            # ---- persistent SBUF tensors -------------------------------
            ring = cpool.tile([128, RCOLS], F32)
            w35sb = cpool.tile([128, 128], F32)
            woutsb = cpool.tile([128, C], F32)
            bo_sb = cpool.tile([1, C], F32)
            ones_sb = cpool.tile([1, 128], F32)
            c0 = cpool.tile([64, BS], F32)
            c1 = cpool.tile([64, BS], F32)
            c_tiles = [c0, c1]

            # chained barriers: each absorbs a few init deps (HW limits
            # sync-wait commands per instruction; barriers chain through
            # the sync engine so earlier ones are implied)
            nc.sync.dma_start(w35sb[0:35, :], w35[:, :])
            nc.sync.dma_start(w35sb[64:99, :], w35[:, :])
            nc.sync.dma_start(woutsb[0:D, :], wout[:, :])
            tc.strict_bb_all_engine_barrier()
            nc.sync.dma_start(woutsb[64:64 + D, :], wout[:, :])
            nc.sync.dma_start(bo_sb[:, :], bout[:, :])
            nc.sync.dma_start(ones_sb[:, :], ones[:, :])
            tc.strict_bb_all_engine_barrier()
            # lead column of ring0: H_init = 0; C state init = 0
            nc.vector.memset(ring[0:32, 0:BS], 0.0)
            nc.vector.memset(c0[32:64, :], 0.0)
            tc.strict_bb_all_engine_barrier()

            def step(j, gb):
                """One LSTM step: reads ring col j, writes H into col j+1."""
                rhs = ring[gb:gb + 35, j * BS:(j + 1) * BS]
                g = gpool.tile([64, 2 * BS], F32)
                nc.tensor.matmul(
                    g[:, 0:BS], lhsT=w35sb[gb:gb + 35, 0:64], rhs=rhs,
                    start=True, stop=True)
                nc.tensor.matmul(
                    g[:, BS:2 * BS], lhsT=w35sb[gb:gb + 35, 64:128], rhs=rhs,
                    start=True, stop=True)
                tt = tpool.tile([64, 2 * BS], F32)
                nc.scalar.activation(tt[:, :], g[:, :], AX.Tanh, scale=0.5)

                u = spool.tile([D, BS], F32, tag="u")
                nc.vector.scalar_tensor_tensor(
                    u[:, :], tt[0:32, 0:BS], 1.0, tt[0:32, BS:2 * BS],
                    op0=OP.add, op1=OP.mult)
                a = spool.tile([D, BS], F32, tag="a")
                nc.vector.scalar_tensor_tensor(
                    a[:, :], tt[32:64, BS:2 * BS], 1.0,
                    c_tiles[j % 2][32:64, :], op0=OP.add, op1=OP.mult)
                c_new = c_tiles[(j + 1) % 2]
                nc.vector.scalar_tensor_tensor(
                    c_new[32:64, :], a[:, :], 0.5, u[:, :],
                    op0=OP.mult, op1=OP.add)
                m = mpool.tile([64, BS], F32)
                nc.scalar.activation(m[32:64, :], c_new[32:64, :], AX.Tanh,
                                     scale=0.5)
                nc.vector.scalar_tensor_tensor(
                    ring[gb:gb + 32, (j + 1) * BS:(j + 2) * BS],
                    tt[32:64, 0:BS], 1.0, m[32:64, :],
                    op0=OP.add, op1=OP.mult)

            def outproj_mm(gb, blk):
                """Project 16 steps (128 ring cols) -> [128, C] psum block."""
                po = opool.tile([128, C], F32)
                nc.tensor.matmul(
                    po[:, :], lhsT=ones_sb[0:1, :], rhs=bo_sb[0:1, :],
                    start=True, stop=False)
                nc.tensor.matmul(
                    po[:, :],
                    lhsT=ring[gb:gb + 32, BS + blk * 128:BS + (blk + 1) * 128],
                    rhs=woutsb[gb:gb + D, :],
                    start=False, stop=True)
                return po

            def outproj_copy(po, blk, osb, q):
                # quarter-sized psum->sbuf copies so the ACT engine's
                # in-order queue never blocks the scan chain for long
                nc.scalar.copy(osb[:, blk * C + q * 64:blk * C + (q + 1) * 64],
                               po[:, q * 64:(q + 1) * 64])

            with tc.For_i(0, ITERS, 1, hint_engines=(
                    ET.PE, ET.Activation, ET.DVE, ET.Pool, ET.SP)) as it:
                def dyn(expr_static, expr_it, size):
                    # bench mode drops the loop-var offset (static APs)
                    if BENCH_ITERS:
                        return ds(expr_static, size)
                    return ds(expr_it, size)

                # refill r3 rows of both rings for this iteration
                nc.sync.dma_start(
                    ring[32:35, 0:HALF * BS],
                    r3x[:, dyn(0, it * (2 * HALF * BS), HALF * BS)])
                nc.sync.dma_start(
                    ring[96:99, 0:HALF * BS],
                    r3x[:, dyn(HALF * BS, it * (2 * HALF * BS) + HALF * BS,
                               HALF * BS)])

                for half, gb in ((0, 0), (1, 64)):
                    osb = obpool.tile([128, NBLK * C], F32)
                    po_pend = None
                    for j in range(HALF):
                        step(j, gb)
                        if j % 4 == 3 and po_pend is not None:
                            blk, po, q = po_pend
                            outproj_copy(po, blk, osb, q)
                            po_pend = (blk, po, q + 1) if q < 3 else None
                        if j % 16 == 15:
                            blk = j // 16
                            po = outproj_mm(gb, blk)
                            po_pend = (blk, po, 0)
                    # drain the last block's copies
                    blk, po, q0 = po_pend
                    for q in range(q0, 4):
                        outproj_copy(po, blk, osb, q)
                    # output DMA per batch row: [16, NBLK, C] each
                    # (partition p = (t%16)*8 + b, free = (blk, c),
                    #  t = t0 + blk*16 + t%16)
                    osb4 = osb[:, :].rearrange(
                        "(t16 b) (blk c) -> t16 b blk c", b=BS, c=C)
                    for b in range(BS):
                        nc.sync.dma_start(
                            out[b, dyn(half * HALF,
                                       it * (2 * HALF) + half * HALF,
                                       HALF), :].rearrange(
                                "(blk t16) c -> t16 blk c", t16=16),
                            osb4[:, b, :, :])
                    # hand the half-boundary H across rings (cross-partition
                    # move -> DMA).  half 0: ring0 tail -> ring1 lead;
                    # half 1: ring1 tail -> ring0 lead (for next iteration).
                    src_gb, dst_gb = (0, 64) if half == 0 else (64, 0)
                    nc.sync.dma_start(
                        ring[dst_gb:dst_gb + 32, 0:BS],
                        ring[src_gb:src_gb + 32, HALF * BS:(HALF + 1) * BS])
    nc.compile()
    return nc


def _prep_host(inputs: dict[str, np.ndarray]):
    """Host-side constants shared by all cores (tiny)."""
    f32 = np.float32
    Wi = np.asarray(inputs["Wi"], f32)
    Wh = np.asarray(inputs["Wh"], f32)
    W_in = np.asarray(inputs["W_in"], f32)
    b_in = np.asarray(inputs["b_in"], f32)
    b_lstm = np.asarray(inputs["b_lstm"], f32)
    bos = np.asarray(inputs["bos"], f32)
    W_out = np.asarray(inputs["W_out"], f32)
    b_out = np.asarray(inputs["b_out"], f32)

    # gate order: source (i,f,g,o) -> target cols (i,o | g,f) so the two
    # M=64 matmuls put every DVE operand pair at one base partition
    perm = np.concatenate([np.arange(0, D), np.arange(3 * D, 4 * D),
                           np.arange(2 * D, 3 * D), np.arange(D, 2 * D)])
    gscale = np.ones(4 * D, f32)
    gscale[2 * D:3 * D] = 2.0  # g-gate block doubled (undoes the 0.5 scale)

    v = (Wi.T @ W_in[0]).astype(f32)[perm] * gscale
    w = (Wi.T @ b_in + b_lstm).astype(f32)[perm] * gscale
    g0 = (Wi.T @ bos + b_lstm).astype(f32)[perm] * gscale
    wh_eff = (0.5 * Wh[:, perm] * gscale[None, :]).astype(f32)
    w35 = np.concatenate([wh_eff, v[None], w[None], g0[None]], 0).astype(f32)
    wout_eff = (0.5 * W_out).astype(f32)
    return w35, wout_eff, b_out.reshape(1, C).astype(f32)


def kernel(**inputs) -> np.ndarray:
    x = np.asarray(inputs["x"])
    assert x.shape == (B, T) and x.dtype == np.int32
    w35, wout_eff, bout = _prep_host(inputs)
    ones = np.ones((1, 128), np.float32)

    xf = (x.astype(np.float32) / np.float32(255.0) - np.float32(0.5))

    # r3[:, t, b] = (xf[b,t-1], 1, 0) for t>=1 ; (0,0,1) at t=0
    in_maps = []
    for core in range(NCORES):
        xs = xf[core * BS:(core + 1) * BS]           # [BS, T]
        r3 = np.zeros((3, T_RUN, BS), np.float32)
        r3[0, 1:, :] = xs[:, :T_RUN - 1].T
        r3[1, 1:, :] = 1.0
        r3[2, 0, :] = 1.0
        in_maps.append({
            "r3x": r3.reshape(3, T_RUN * BS), "w35": w35,
            "wout": wout_eff, "bout": bout, "ones": ones,
        })

    nc = build_bass()
    res = run_bass_kernel_spmd(nc, in_maps, core_ids=list(range(NCORES)),
                               trace=TRACE)
    global LAST_RESULTS
    LAST_RESULTS = res
    outs = [res.results[i]["out"] for i in range(NCORES)]
    return np.concatenate(outs, axis=0)


TRACE = False           # set True (e.g. from test.py) to capture an NTFF trace
LAST_RESULTS = None     # BassKernelResults of the last kernel() call


if __name__ == "__main__":
    rng = np.random.default_rng(0)
    ins = {
        "x": rng.integers(0, C, size=(B, T), dtype=np.int32),
        "bos": rng.normal(size=(D,)).astype(np.float32) * 0.01,
        "W_in": rng.normal(size=(1, D)).astype(np.float32),
        "b_in": np.zeros((D,), np.float32),
        "Wi": rng.normal(size=(D, 4 * D)).astype(np.float32) / np.sqrt(D),
        "Wh": rng.normal(size=(D, 4 * D)).astype(np.float32) / np.sqrt(D),
        "b_lstm": np.zeros((4 * D,), np.float32),
        "W_out": rng.normal(size=(D, C)).astype(np.float32) / np.sqrt(D),
        "b_out": np.zeros((C,), np.float32),
    }
    o = kernel(**ins)
    print("kernel out", o.shape, o.dtype, float(np.abs(o).max()))
